# revision 24
# baseline (speedup 1.0000x reference)
"""Trainium2 Bass kernel for the bidirectional Mamba MixerModel problem.

Contract: kernel(**inputs) takes the FULL inputs from setup_inputs() and
returns the FULL (B, L, D_MODEL) output.  Internally the work is sharded
over 8 NeuronCores: tensor-parallel over d_inner (128 channels per core),
with per-block AllReduces for the x_dbl projection and the out-projection
partial sums.

v2: bf16 matmuls/activations/collectives, fused stats+in-proj+conv input
pass, full-length (2048) scans without carries, and the per-state B/C row
broadcasts done by partition-broadcast DMA so both scan multiplies run as
bf16 2x DVE ops.  dA stays fp32 (decay-rate precision), PSUM stays fp32.
"""
import sys
import numpy as np

sys.path.insert(0, "/opt/trn_rl_repo")

import ml_dtypes  # noqa: E402
import concourse.bass as bass  # noqa: E402,F401
import concourse.bacc as bacc  # noqa: E402
import concourse.tile as tile  # noqa: E402
from concourse import mybir  # noqa: E402
from concourse import bass_utils  # noqa: E402

F32 = mybir.dt.float32
BF16 = mybir.dt.bfloat16
Alu = mybir.AluOpType
Act = mybir.ActivationFunctionType

B, L, D, DI = 2, 2048, 512, 1024
NST, KCONV, RDT, NB = 16, 4, 32, 4
NCORES = 8
DS = DI // NCORES          # 128 channels per core
T = B * L                  # 4096 tokens, token index = b*L + l
CH = 512                   # token chunk (1 PSUM bank at fp32)
NCH = L // CH              # 4 chunks per batch
NG = D // 128              # 4 partition groups of the model dim
EPS = 1e-5

_PROGRAM_CACHE = {}


def _build_program(has_lnb: bool, has_nfb: bool):
    nc = bacc.Bacc("TRN2", target_bir_lowering=False, debug=False,
                   enable_asserts=False, num_devices=NCORES)

    tensors = {}
    tensors["xT"] = nc.dram_tensor("xT", [D, T], F32, kind="ExternalInput")
    tensors["wi"] = nc.dram_tensor("wi", [NB, 128, 1024], F32, kind="ExternalInput")
    tensors["negrs"] = nc.dram_tensor("negrs", [NB, 1, 256], F32, kind="ExternalInput")
    tensors["biasin"] = nc.dram_tensor("biasin", [NB, 128, 2], F32, kind="ExternalInput")
    tensors["convd"] = nc.dram_tensor("convd", [NB, 128, KCONV * 128], BF16, kind="ExternalInput")
    tensors["convb"] = nc.dram_tensor("convb", [NB, 128, 1], F32, kind="ExternalInput")
    tensors["wxT"] = nc.dram_tensor("wxT", [NB, 128, 64], BF16, kind="ExternalInput")
    tensors["wdtT"] = nc.dram_tensor("wdtT", [NB, 32, 128], BF16, kind="ExternalInput")
    tensors["bdt"] = nc.dram_tensor("bdt", [NB, 1, 128], BF16, kind="ExternalInput")
    tensors["acols"] = nc.dram_tensor("acols", [NB, 128, NST], F32, kind="ExternalInput")
    tensors["dpcol"] = nc.dram_tensor("dpcol", [NB, 128, 1], F32, kind="ExternalInput")
    tensors["woT"] = nc.dram_tensor("woT", [NB, 128, 512], F32, kind="ExternalInput")
    tensors["nfw"] = nc.dram_tensor("nfw", [128, NG], F32, kind="ExternalInput")
    tensors["nfb"] = nc.dram_tensor("nfb", [128, NG], F32, kind="ExternalInput")
    tensors["identin"] = nc.dram_tensor("identin", [128, 128], BF16, kind="ExternalInput")
    tensors["outT"] = nc.dram_tensor("outT", [D, T], F32, kind="ExternalOutput")
    tensors["rstd_scr"] = nc.dram_tensor("rstd_scr", [B, L], F32, kind="Internal")

    xdbl_in, xdbl_out, op_in, op_out = [], [], [], []
    for i in range(NB):
        xi_b, xo_b, oi_b, oo_b = [], [], [], []
        for b in range(B):
            xi_b.append(nc.dram_tensor(f"xdbl_in_{i}_{b}", [64, L], BF16,
                                       kind="Internal"))
            xo_b.append(nc.dram_tensor(f"xdbl_out_{i}_{b}", [64, L], BF16,
                                       kind="Internal", addr_space="Shared"))
            oi_b.append(nc.dram_tensor(f"op_in_{i}_{b}", [D, L], F32,
                                       kind="Internal"))
            oo_b.append(nc.dram_tensor(f"op_out_{i}_{b}", [D, L], F32,
                                       kind="Internal", addr_space="Shared"))
        xdbl_in.append(xi_b); xdbl_out.append(xo_b)
        op_in.append(oi_b); op_out.append(oo_b)
    tensors["xdbl_in"], tensors["xdbl_out"] = xdbl_in, xdbl_out
    tensors["op_in"], tensors["op_out"] = op_in, op_out

    with tile.TileContext(nc) as tc:
        _emit(nc, tc, tensors, has_lnb, has_nfb)

    nc.compile()
    return nc


def _emit(nc, tc, Tn, has_lnb, has_nfb):
    import contextlib
    RG = [list(range(NCORES))]
    xdbl_in, xdbl_out = Tn["xdbl_in"], Tn["xdbl_out"]
    op_in, op_out = Tn["op_in"], Tn["op_out"]

    ctx = contextlib.ExitStack()
    with ctx:
        consts = ctx.enter_context(tc.tile_pool(name="consts", bufs=1))
        wpool = ctx.enter_context(tc.tile_pool(name="wpool", bufs=2))
        xin = ctx.enter_context(tc.tile_pool(name="xin", bufs=2))
        xsqp = ctx.enter_context(tc.tile_pool(name="xsqp", bufs=2))
        small = ctx.enter_context(tc.tile_pool(name="small", bufs=2))
        stats = ctx.enter_context(tc.tile_pool(name="stats", bufs=2))
        bigs = ctx.enter_context(tc.tile_pool(name="bigs", bufs=1))
        dpool = ctx.enter_context(tc.tile_pool(name="dpool", bufs=2))
        spool = ctx.enter_context(tc.tile_pool(name="spool", bufs=2))
        bcpool = ctx.enter_context(tc.tile_pool(name="bcpool", bufs=2))
        evac = ctx.enter_context(tc.tile_pool(name="evac", bufs=2))
        ps_mm = ctx.enter_context(tc.tile_pool(name="ps_mm", bufs=2, space="PSUM"))
        ps_st = ctx.enter_context(tc.tile_pool(name="ps_st", bufs=2, space="PSUM"))
        ps_y = ctx.enter_context(tc.tile_pool(name="ps_y", bufs=1, space="PSUM"))

        ident = consts.tile([128, 128], BF16, tag="ident")
        nc.sync.dma_start(out=ident[:], in_=Tn["identin"].ap())
        ones1 = consts.tile([1, 128], F32, tag="ones1")
        nc.vector.memset(ones1[:], 1.0)
        onescol = consts.tile([128, 1], F32, tag="onescol")
        nc.vector.memset(onescol[:], 1.0)
        onescol16 = consts.tile([128, 1], BF16, tag="onescol16")
        nc.vector.memset(onescol16[:], 1.0)
        ones_row = consts.tile([1, CH], BF16, tag="ones_row")
        nc.vector.memset(ones_row[:], 1.0)
        nfw_sb = consts.tile([128, NG], F32, tag="nfw")
        nc.sync.dma_start(out=nfw_sb[:], in_=Tn["nfw"].ap())
        nfb_sb = consts.tile([128, NG], F32, tag="nfb")
        nc.sync.dma_start(out=nfb_sb[:], in_=Tn["nfb"].ap())
        eps_sb = consts.tile([128, 1], F32, tag="eps")
        nc.vector.memset(eps_sb[:], EPS)

        def mm(out, lhsT, rhs, **kw):
            nc.tensor.matmul(out, lhsT=lhsT, rhs=rhs, **kw)

        def src_ap(i, b, g, t0, t1):
            """Block-i input (already flipped), batch b, feature group g."""
            if i == 0:
                return Tn["xT"].ap()[128 * g:128 * (g + 1), b * L + t0: b * L + t1]
            return op_out[i - 1][b].ap()[128 * g:128 * (g + 1), t0:t1]

        for i in range(NB):
            # ---------------- per-block weights ----------------
            wi_sb = wpool.tile([128, 1024], F32, tag="wi", bufs=1)
            nc.sync.dma_start(out=wi_sb[:], in_=Tn["wi"].ap()[i])
            negrs_sb = wpool.tile([1, 256], F32, tag="negrs", bufs=1)
            nc.sync.dma_start(out=negrs_sb[:], in_=Tn["negrs"].ap()[i])
            convd_sb = wpool.tile([128, KCONV * 128], BF16, tag="convd", bufs=1)
            nc.sync.dma_start(out=convd_sb[:], in_=Tn["convd"].ap()[i])
            convb_sb = wpool.tile([128, 1], F32, tag="convb")
            nc.sync.dma_start(out=convb_sb[:], in_=Tn["convb"].ap()[i])
            wx_sb = wpool.tile([128, 64], BF16, tag="wx", bufs=1)
            nc.sync.dma_start(out=wx_sb[:], in_=Tn["wxT"].ap()[i])
            wdt_sb = wpool.tile([32, 128], BF16, tag="wdt", bufs=1)
            nc.sync.dma_start(out=wdt_sb[:], in_=Tn["wdtT"].ap()[i])
            bdt_sb = wpool.tile([1, 128], BF16, tag="bdt", bufs=1)
            nc.sync.dma_start(out=bdt_sb[:], in_=Tn["bdt"].ap()[i])
            acols_sb = wpool.tile([128, NST], F32, tag="acols", bufs=1)
            nc.sync.dma_start(out=acols_sb[:], in_=Tn["acols"].ap()[i])
            dpcol_sb = wpool.tile([128, 1], F32, tag="dpcol", bufs=1)
            nc.sync.dma_start(out=dpcol_sb[:], in_=Tn["dpcol"].ap()[i])
            wo_sb = wpool.tile([128, 512], F32, tag="wo", bufs=1)
            nc.sync.dma_start(out=wo_sb[:], in_=Tn["woT"].ap()[i])
            if has_lnb:
                biasin_sb = wpool.tile([128, 2], F32, tag="biasin")
                nc.sync.dma_start(out=biasin_sb[:], in_=Tn["biasin"].ap()[i])

            # ---- fused stats + in-proj + conv, per batch ----
            xipad = [bigs.tile([128, L + 3], BF16, tag=f"xipad{b}",
                               name=f"xipad{b}") for b in range(B)]
            sz = [bigs.tile([128, L], F32, tag=f"sz{b}", name=f"sz{b}")
                  for b in range(B)]
            for b in range(B):
                nc.vector.memset(xipad[b][:, 0:3], 0.0)
            for b in range(B):
                # full-L group tiles, loaded once per (block, batch)
                xg_full = []
                for g in range(NG):
                    xg = xin.tile([128, L], F32, tag=f"xg{g}", bufs=1, name=f"xg{g}")
                    nc.sync.dma_start(out=xg[:], in_=src_ap(i, b, g, 0, L))
                    xg_full.append(xg)
                for c in range(NCH):
                    t0 = c * CH
                    s1_ps = ps_st.tile([1, CH], F32, tag="st")
                    s2_ps = ps_st.tile([1, CH], F32, tag="st")
                    for g in range(NG):
                        xsq = xsqp.tile([128, CH], BF16, tag="xsq",
                                        name="xsq")
                        nc.scalar.square(out=xsq[:],
                                         in_=xg_full[g][:, t0:t0 + CH])
                        mm(s1_ps[:], lhsT=onescol[:],
                           rhs=xg_full[g][:, t0:t0 + CH],
                           start=(g == 0), stop=(g == NG - 1))
                        mm(s2_ps[:], lhsT=onescol16[:],
                           rhs=xsq[:],
                           start=(g == 0), stop=(g == NG - 1))
                    s1_row = stats.tile([1, CH], F32, tag="s1",
                                        name="s1_row")
                    nc.scalar.copy(out=s1_row[:], in_=s1_ps[:])
                    mu2 = small.tile([1, CH], F32, tag="mu2")
                    nc.scalar.activation(out=mu2[:], in_=s1_ps[:],
                                         func=Act.Square, scale=1.0 / D)
                    var_row = stats.tile([1, CH], F32, tag="lnv",
                                         name="lnv_row")
                    nc.vector.scalar_tensor_tensor(
                        out=var_row[:], in0=s2_ps[:],
                        scalar=1.0 / D, in1=mu2[:], op0=Alu.mult,
                        op1=Alu.subtract)
                    nc.scalar.activation(out=var_row[:], in_=var_row[:],
                                         func=Act.Ln, bias=eps_sb[:1, :])
                    rstd_row = small.tile([1, CH], F32, tag="rstds",
                                          name="rstd_row")
                    nc.scalar.activation(out=rstd_row[:], in_=var_row[:],
                                         func=Act.Exp, scale=-0.5)
                    nc.sync.dma_start(
                        out=Tn["rstd_scr"].ap()[b:b + 1, t0:t0 + CH],
                        in_=rstd_row[:])
                    rbc = small.tile([128, CH], F32, tag="rbc")
                    nc.sync.dma_start(
                        out=rbc[:],
                        in_=Tn["rstd_scr"].ap()[b:b + 1, t0:t0 + CH]
                            .partition_broadcast(128))
                    for grp in range(2):  # 0 = xi, 1 = z
                        xz_ps = ps_mm.tile([128, CH], F32, tag="mm")
                        for k in range(4):
                            lh = wi_sb[:, (grp * 4 + k) * 128:
                                       (grp * 4 + k + 1) * 128]
                            mm(xz_ps[:], lhsT=lh,
                               rhs=xg_full[k][:, t0:t0 + CH],
                               start=(k == 0), stop=False)
                        mm(xz_ps[:],
                           lhsT=negrs_sb[:, grp * 128:(grp + 1) * 128],
                           rhs=s1_row[:], start=False, stop=True)
                        if grp == 0:
                            dest = xipad[b][:, 3 + t0: 3 + t0 + CH]
                        else:
                            dest = sz[b][:, t0: t0 + CH]
                        nc.vector.tensor_mul(out=dest, in0=xz_ps[:],
                                             in1=rbc[:])
                        if has_lnb:
                            nc.vector.tensor_scalar_add(
                                out=dest, in0=dest,
                                scalar1=biasin_sb[:, grp:grp + 1])
                    cv_ps = ps_mm.tile([128, CH], F32, tag="mm")
                    for kk in range(KCONV):
                        mm(cv_ps[:],
                           lhsT=convd_sb[:, kk * 128:(kk + 1) * 128],
                           rhs=xipad[b][:, t0 + kk: t0 + kk + CH],
                           start=(kk == 0), stop=(kk == KCONV - 1))
                    nc.scalar.activation(out=xipad[b][:, t0:t0 + CH],
                                         in_=cv_ps[:], func=Act.Identity,
                                         bias=convb_sb[:])
            # ---- silus via exp/ln set (avoids act-table switches):
            # silu(x) = x * exp(-ln(1 + exp(-x)))
            for b in range(B):
                for t_sb in (xipad[b][:, 0:L], sz[b][:]):
                    sg = dpool.tile([128, L], F32, tag="esb", bufs=1,
                                    name="sg")
                    nc.scalar.activation(out=sg[:], in_=t_sb,
                                         func=Act.Exp, scale=-1.0)
                    nc.scalar.activation(out=sg[:], in_=sg[:],
                                         func=Act.Ln, bias=1.0)
                    nc.scalar.activation(out=sg[:], in_=sg[:],
                                         func=Act.Exp, scale=-1.0)
                    nc.vector.tensor_mul(out=t_sb, in0=t_sb, in1=sg[:])
            # ---- Wx projection + AllReduce per batch ----
            for b in range(B):
                for c in range(NCH):
                    t0 = c * CH
                    wx_ps = ps_mm.tile([64, CH], F32, tag="mm")
                    mm(wx_ps[:], lhsT=wx_sb[:], rhs=xipad[b][:, t0:t0 + CH],
                       start=True, stop=True)
                    wxe = small.tile([64, CH], BF16, tag="mu2", name="wxe")
                    nc.scalar.copy(out=wxe[:], in_=wx_ps[:])
                    nc.sync.dma_start(out=xdbl_in[i][b].ap()[:, t0:t0 + CH],
                                      in_=wxe[:])
                nc.gpsimd.collective_compute(
                    "AllReduce", Alu.add, replica_groups=RG,
                    ins=[xdbl_in[i][b].ap()], outs=[xdbl_out[i][b].ap()])

            # ---- phase D: dt (softplus), full-L scan per state ----
            for b in range(B):
                xdbl_sb = dpool.tile([64, L], BF16, tag="xdbl", bufs=1, name="xdbl")
                nc.sync.dma_start(out=xdbl_sb[:],
                                  in_=xdbl_out[i][b].ap())
                dtraw = dpool.tile([128, L], F32, tag="dtraw", bufs=1, name="dtraw")
                esb = dpool.tile([128, L], F32, tag="esb", bufs=1, name="esb")
                for c in range(NCH):
                    t0 = c * CH
                    dt_ps = ps_mm.tile([128, CH], F32, tag="mm",
                                       name="dt_ps")
                    mm(dt_ps[:], lhsT=wdt_sb[:],
                       rhs=xdbl_sb[0:32, t0:t0 + CH],
                       start=True, stop=False)
                    mm(dt_ps[:], lhsT=bdt_sb[:], rhs=ones_row[:],
                       start=False, stop=True)
                    nc.scalar.activation(out=esb[:, t0:t0 + CH],
                                         in_=dt_ps[:],
                                         func=Act.Exp, scale=-1.0)
                    nc.scalar.copy(out=dtraw[:, t0:t0 + CH], in_=dt_ps[:])
                nc.scalar.activation(out=esb[:], in_=esb[:],
                                     func=Act.Ln, bias=1.0)
                dt16 = dpool.tile([128, L], F32, tag="dt16", bufs=1, name="dt16")
                nc.vector.tensor_add(out=dt16[:], in0=dtraw[:], in1=esb[:])
                dtx = dpool.tile([128, L], BF16, tag="dtx", bufs=1, name="dtx")
                nc.vector.tensor_mul(out=dtx[:], in0=dt16[:],
                                     in1=xipad[b][:, 0:L])

                y_ps = ps_y.tile([128, L], F32, tag="y")
                for n in range(NST):
                    bbc = bcpool.tile([128, L], BF16, tag="bbc", name="bbc")
                    nc.sync.dma_start(
                        out=bbc[:],
                        in_=xdbl_out[i][b].ap()[32 + n:33 + n, :]
                            .partition_broadcast(128))
                    cbc = bcpool.tile([128, L], BF16, tag="cbc", name="cbc")
                    nc.sync.dma_start(
                        out=cbc[:],
                        in_=xdbl_out[i][b].ap()[48 + n:49 + n, :]
                            .partition_broadcast(128))
                    dA = spool.tile([128, L], F32, tag="dA", bufs=2)
                    nc.scalar.activation(out=dA[:], in_=dt16[:],
                                         func=Act.Exp,
                                         scale=acols_sb[:, n:n + 1])
                    dBu = spool.tile([128, L], BF16, tag="dBu", bufs=1)
                    nc.vector.tensor_mul(out=dBu[:], in0=dtx[:], in1=bbc[:])
                    h = spool.tile([128, L], BF16, tag="h", bufs=1)
                    nc.vector.tensor_tensor_scan(h[:], dA[:], dBu[:],
                                                 0.0, op0=Alu.mult,
                                                 op1=Alu.add)
                    yterm = spool.tile([128, L], BF16, tag="yterm", bufs=1)
                    nc.vector.tensor_mul(out=yterm[:], in0=h[:], in1=cbc[:])
                    for c in range(NCH):
                        t0 = c * CH
                        mm(y_ps[:, t0:t0 + CH], lhsT=ident[:],
                           rhs=yterm[:, t0:t0 + CH],
                           start=(n == 0), stop=(n == NST - 1))
                for c in range(NCH):
                    t0 = c * CH
                    yg = evac.tile([128, CH], F32, tag="yg", bufs=1)
                    nc.vector.scalar_tensor_tensor(
                        out=yg[:], in0=xipad[b][:, t0:t0 + CH],
                        scalar=dpcol_sb[:], in1=y_ps[:, t0:t0 + CH],
                        op0=Alu.mult, op1=Alu.add)
                    nc.vector.tensor_mul(out=yg[:], in0=yg[:],
                                         in1=sz[b][:, t0: t0 + CH])
                    ft0 = L - (c + 1) * CH
                    for g in range(NG):
                        op_ps = ps_mm.tile([128, CH], F32, tag="mm")
                        mm(op_ps[:], lhsT=wo_sb[:, g * 128:(g + 1) * 128],
                           rhs=yg[:], start=True, stop=True)
                        og = evac.tile([128, CH], F32, tag="og")
                        nc.scalar.copy(out=og[:, ::-1], in_=op_ps[:])
                        nc.sync.dma_start(
                            out=op_in[i][b].ap()[g * 128:(g + 1) * 128,
                                                 ft0:ft0 + CH],
                            in_=og[:])
                nc.gpsimd.collective_compute(
                    "AllReduce", Alu.add, replica_groups=RG,
                    ins=[op_in[i][b].ap()], outs=[op_out[i][b].ap()])

        # ---------------- final layernorm (token-replicated) ----------------
        for b in range(B):
            for c in range(NCH):
                t0 = c * CH
                xg_tiles = []
                for g in range(NG):
                    xg = xin.tile([128, CH], F32, tag="xgf", bufs=5,
                                  name="xgf")
                    nc.sync.dma_start(
                        out=xg[:],
                        in_=src_ap(NB, b, g, t0, t0 + CH))
                    xg_tiles.append(xg)
                s1_ps = ps_st.tile([1, CH], F32, tag="st")
                s2_ps = ps_st.tile([1, CH], F32, tag="st")
                for g in range(NG):
                    xsq = small.tile([128, CH], BF16, tag="xsqf",
                                     name="xsqf")
                    nc.scalar.square(out=xsq[:], in_=xg_tiles[g][:])
                    mm(s1_ps[:], lhsT=onescol[:], rhs=xg_tiles[g][:],
                       start=(g == 0), stop=(g == NG - 1))
                    mm(s2_ps[:], lhsT=onescol16[:], rhs=xsq[:],
                       start=(g == 0), stop=(g == NG - 1))
                m_row = small.tile([1, CH], F32, tag="m_row")
                nc.vector.tensor_scalar_mul(out=m_row[:], in0=s1_ps[:],
                                            scalar1=1.0 / D)
                mu2 = small.tile([1, CH], F32, tag="mu2f")
                nc.scalar.activation(out=mu2[:], in_=s1_ps[:],
                                     func=Act.Square, scale=1.0 / D)
                var_row = small.tile([1, CH], F32, tag="var")
                nc.vector.scalar_tensor_tensor(
                    out=var_row[:], in0=s2_ps[:], scalar=1.0 / D, in1=mu2[:],
                    op0=Alu.mult, op1=Alu.subtract)
                rstd_row = small.tile([1, CH], F32, tag="rstdf",
                                      name="rstd_row")
                nc.scalar.activation(out=var_row[:], in_=var_row[:],
                                     func=Act.Ln, bias=eps_sb[:1, :])
                nc.scalar.activation(out=rstd_row[:], in_=var_row[:],
                                     func=Act.Exp, scale=-0.5)
                mbc_ps = ps_mm.tile([128, CH], F32, tag="mm")
                mm(mbc_ps[:], lhsT=ones1[:], rhs=m_row[:],
                   start=True, stop=True)
                nc.sync.dma_start(
                    out=Tn["rstd_scr"].ap()[b:b + 1, t0:t0 + CH],
                    in_=rstd_row[:])
                rbc = small.tile([128, CH], F32, tag="rbcf")
                nc.sync.dma_start(
                    out=rbc[:],
                    in_=Tn["rstd_scr"].ap()[b:b + 1, t0:t0 + CH]
                        .partition_broadcast(128))
                for g in range(NG):
                    t1_sb = small.tile([128, CH], F32, tag="t1f",
                                       name="t1_sb")
                    nc.vector.tensor_sub(out=t1_sb[:],
                                         in0=xg_tiles[g][:],
                                         in1=mbc_ps[:])
                    o_sb = evac.tile([128, CH], F32, tag="og", name="o_sb")
                    nc.vector.scalar_tensor_tensor(
                        out=o_sb[:], in0=t1_sb[:], scalar=nfw_sb[:, g:g + 1],
                        in1=rbc[:], op0=Alu.mult, op1=Alu.mult)
                    if has_nfb:
                        nc.vector.tensor_scalar_add(
                            out=o_sb[:], in0=o_sb[:],
                            scalar1=nfb_sb[:, g:g + 1])
                    nc.sync.dma_start(
                        out=Tn["outT"].ap()[g * 128:(g + 1) * 128,
                                            b * L + t0: b * L + t0 + CH],
                        in_=o_sb[:])


def _host_prep(inputs):
    x = np.asarray(inputs["x"], np.float32)
    ln_w = np.asarray(inputs["ln_w"], np.float32)
    ln_b = np.asarray(inputs["ln_b"], np.float32)
    W_in = np.asarray(inputs["W_in"], np.float32)
    conv_w = np.asarray(inputs["conv_w"], np.float32)
    conv_b = np.asarray(inputs["conv_b"], np.float32)
    W_x = np.asarray(inputs["W_x"], np.float32)
    W_dt = np.asarray(inputs["W_dt"], np.float32)
    b_dt = np.asarray(inputs["b_dt"], np.float32)
    A_log = np.asarray(inputs["A_log"], np.float32)
    D_p = np.asarray(inputs["D_p"], np.float32)
    W_out = np.asarray(inputs["W_out"], np.float32)
    normf_w = np.asarray(inputs["normf_w"], np.float32)
    normf_b = np.asarray(inputs["normf_b"], np.float32)

    bf = ml_dtypes.bfloat16
    xT = np.ascontiguousarray(x.transpose(2, 0, 1).reshape(D, T))
    A = -np.exp(A_log)  # (NB, DI, NST)

    in_maps = []
    for k in range(NCORES):
        sl = slice(DS * k, DS * (k + 1))
        wi_arr = np.zeros((NB, 128, 1024), np.float32)
        negrs_arr = np.zeros((NB, 1, 256), np.float32)
        biasin_arr = np.zeros((NB, 128, 2), np.float32)
        convd_arr = np.zeros((NB, 128, KCONV * 128), np.float32)
        convb_arr = np.zeros((NB, 128, 1), np.float32)
        wx_arr = np.zeros((NB, 128, 64), np.float32)
        wdt_arr = np.zeros((NB, 32, 128), np.float32)
        bdt_arr = np.zeros((NB, 1, 128), np.float32)
        acols_arr = np.zeros((NB, 128, NST), np.float32)
        dpcol_arr = np.zeros((NB, 128, 1), np.float32)
        wo_arr = np.zeros((NB, 128, 512), np.float32)
        for i in range(NB):
            Wf = W_in[i] * ln_w[i][None, :]          # (2DI, D)
            rows = [np.arange(DS * k, DS * (k + 1)),
                    np.arange(DI + DS * k, DI + DS * (k + 1))]
            for grp in range(2):
                Wg = Wf[rows[grp], :]                # (128, 512)
                lhsT = Wg.T.reshape(4, 128, 128)     # [kc, p, m]
                for kc in range(4):
                    wi_arr[i, :, (grp * 4 + kc) * 128:(grp * 4 + kc + 1) * 128] = \
                        lhsT[kc]
                negrs_arr[i, 0, grp * 128:(grp + 1) * 128] = -Wg.sum(1) / D
                biasin_arr[i, :, grp] = W_in[i][rows[grp], :] @ ln_b[i]
            for kk in range(KCONV):
                np.fill_diagonal(
                    convd_arr[i, :, kk * 128:(kk + 1) * 128],
                    conv_w[i, sl, kk])
            convb_arr[i, :, 0] = conv_b[i, sl]
            wx_arr[i] = W_x[i][:, sl].T              # (128, 64)
            wdt_arr[i] = W_dt[i][sl, :].T            # (32, 128)
            bdt_arr[i, 0, :] = b_dt[i, sl]
            acols_arr[i] = A[i, sl, :]
            dpcol_arr[i, :, 0] = D_p[i, sl]
            wo_arr[i] = W_out[i][:, sl].T            # (128, 512)
        in_maps.append({
            "xT": xT,
            "wi": wi_arr, "negrs": negrs_arr,
            "biasin": biasin_arr,
            "convd": convd_arr.astype(bf), "convb": convb_arr,
            "wxT": wx_arr.astype(bf), "wdtT": wdt_arr.astype(bf),
            "bdt": bdt_arr.astype(bf),
            "acols": acols_arr, "dpcol": dpcol_arr,
            "woT": wo_arr,
            "nfw": np.ascontiguousarray(normf_w.reshape(NG, 128).T),
            "nfb": np.ascontiguousarray(normf_b.reshape(NG, 128).T),
            "identin": np.eye(128, dtype=np.float32).astype(bf),
        })
    has_lnb = bool(np.any(ln_b != 0.0))
    has_nfb = bool(np.any(normf_b != 0.0))
    return in_maps, has_lnb, has_nfb


def _get_program(has_lnb, has_nfb):
    key = (has_lnb, has_nfb)
    if key not in _PROGRAM_CACHE:
        _PROGRAM_CACHE[key] = _build_program(has_lnb, has_nfb)
    return _PROGRAM_CACHE[key]


def kernel(**inputs) -> np.ndarray:
    in_maps, has_lnb, has_nfb = _host_prep(inputs)
    nc = _get_program(has_lnb, has_nfb)
    res = bass_utils.run_bass_kernel_spmd(nc, in_maps,
                                          core_ids=list(range(NCORES)))
    out_T = res.results[0]["outT"]                   # (512, 4096)
    out = out_T.reshape(D, B, L).transpose(1, 2, 0)  # (B, L, D)
    return np.ascontiguousarray(out.astype(np.float32))


# revision 25
# speedup vs baseline: 1.0162x; 1.0162x over previous
"""Trainium2 Bass kernel for the bidirectional Mamba MixerModel problem.

Contract: kernel(**inputs) takes the FULL inputs from setup_inputs() and
returns the FULL (B, L, D_MODEL) output.  Internally the work is sharded
over 8 NeuronCores: tensor-parallel over d_inner (128 channels per core),
with per-block AllReduces for the x_dbl projection and the out-projection
partial sums.

v2: bf16 matmuls/activations/collectives, fused stats+in-proj+conv input
pass, full-length (2048) scans without carries, and the per-state B/C row
broadcasts done by partition-broadcast DMA so both scan multiplies run as
bf16 2x DVE ops.  dA stays fp32 (decay-rate precision), PSUM stays fp32.
"""
import sys
import numpy as np

sys.path.insert(0, "/opt/trn_rl_repo")

import ml_dtypes  # noqa: E402
import concourse.bass as bass  # noqa: E402,F401
import concourse.bacc as bacc  # noqa: E402
import concourse.tile as tile  # noqa: E402
from concourse import mybir  # noqa: E402
from concourse import bass_utils  # noqa: E402

F32 = mybir.dt.float32
BF16 = mybir.dt.bfloat16
Alu = mybir.AluOpType
Act = mybir.ActivationFunctionType

B, L, D, DI = 2, 2048, 512, 1024
NST, KCONV, RDT, NB = 16, 4, 32, 4
NCORES = 8
DS = DI // NCORES          # 128 channels per core
T = B * L                  # 4096 tokens, token index = b*L + l
CH = 512                   # token chunk (1 PSUM bank at fp32)
NCH = L // CH              # 4 chunks per batch
NG = D // 128              # 4 partition groups of the model dim
EPS = 1e-5

_PROGRAM_CACHE = {}


def _build_program(has_lnb: bool, has_nfb: bool):
    nc = bacc.Bacc("TRN2", target_bir_lowering=False, debug=False,
                   enable_asserts=False, num_devices=NCORES)

    tensors = {}
    tensors["xT"] = nc.dram_tensor("xT", [D, T], F32, kind="ExternalInput")
    tensors["wi"] = nc.dram_tensor("wi", [NB, 128, 1024], F32, kind="ExternalInput")
    tensors["negrs"] = nc.dram_tensor("negrs", [NB, 1, 256], F32, kind="ExternalInput")
    tensors["biasin"] = nc.dram_tensor("biasin", [NB, 128, 2], F32, kind="ExternalInput")
    tensors["convd"] = nc.dram_tensor("convd", [NB, 128, KCONV * 128], BF16, kind="ExternalInput")
    tensors["convb"] = nc.dram_tensor("convb", [NB, 128, 1], F32, kind="ExternalInput")
    tensors["wxT"] = nc.dram_tensor("wxT", [NB, 128, 64], BF16, kind="ExternalInput")
    tensors["wdtT"] = nc.dram_tensor("wdtT", [NB, 32, 128], BF16, kind="ExternalInput")
    tensors["bdt"] = nc.dram_tensor("bdt", [NB, 1, 128], BF16, kind="ExternalInput")
    tensors["acols"] = nc.dram_tensor("acols", [NB, 128, NST], F32, kind="ExternalInput")
    tensors["dpcol"] = nc.dram_tensor("dpcol", [NB, 128, 1], F32, kind="ExternalInput")
    tensors["woT"] = nc.dram_tensor("woT", [NB, 128, 512], F32, kind="ExternalInput")
    tensors["nfw"] = nc.dram_tensor("nfw", [128, NG], F32, kind="ExternalInput")
    tensors["nfb"] = nc.dram_tensor("nfb", [128, NG], F32, kind="ExternalInput")
    tensors["identin"] = nc.dram_tensor("identin", [128, 128], BF16, kind="ExternalInput")
    tensors["outT"] = nc.dram_tensor("outT", [D, T], F32, kind="ExternalOutput")
    tensors["rstd_scr"] = nc.dram_tensor("rstd_scr", [B, L], F32, kind="Internal")

    xdbl_in, xdbl_out, op_in, op_out = [], [], [], []
    for i in range(NB):
        xi_b, xo_b, oi_b, oo_b = [], [], [], []
        for b in range(B):
            xi_b.append(nc.dram_tensor(f"xdbl_in_{i}_{b}", [64, L], BF16,
                                       kind="Internal"))
            xo_b.append(nc.dram_tensor(f"xdbl_out_{i}_{b}", [64, L], BF16,
                                       kind="Internal", addr_space="Shared"))
            oi_b.append(nc.dram_tensor(f"op_in_{i}_{b}", [D, L], F32,
                                       kind="Internal"))
            oo_b.append(nc.dram_tensor(f"op_out_{i}_{b}", [D, L], F32,
                                       kind="Internal", addr_space="Shared"))
        xdbl_in.append(xi_b); xdbl_out.append(xo_b)
        op_in.append(oi_b); op_out.append(oo_b)
    tensors["xdbl_in"], tensors["xdbl_out"] = xdbl_in, xdbl_out
    tensors["op_in"], tensors["op_out"] = op_in, op_out

    with tile.TileContext(nc) as tc:
        _emit(nc, tc, tensors, has_lnb, has_nfb)

    nc.compile()
    return nc


def _emit(nc, tc, Tn, has_lnb, has_nfb):
    import contextlib
    RG = [list(range(NCORES))]
    xdbl_in, xdbl_out = Tn["xdbl_in"], Tn["xdbl_out"]
    op_in, op_out = Tn["op_in"], Tn["op_out"]

    ctx = contextlib.ExitStack()
    with ctx:
        consts = ctx.enter_context(tc.tile_pool(name="consts", bufs=1))
        wpool = ctx.enter_context(tc.tile_pool(name="wpool", bufs=2))
        xin = ctx.enter_context(tc.tile_pool(name="xin", bufs=2))
        xsqp = ctx.enter_context(tc.tile_pool(name="xsqp", bufs=2))
        small = ctx.enter_context(tc.tile_pool(name="small", bufs=2))
        stats = ctx.enter_context(tc.tile_pool(name="stats", bufs=2))
        bigs = ctx.enter_context(tc.tile_pool(name="bigs", bufs=1))
        dpool = ctx.enter_context(tc.tile_pool(name="dpool", bufs=2))
        spool = ctx.enter_context(tc.tile_pool(name="spool", bufs=2))
        bcpool = ctx.enter_context(tc.tile_pool(name="bcpool", bufs=2))
        evac = ctx.enter_context(tc.tile_pool(name="evac", bufs=2))
        ps_mm = ctx.enter_context(tc.tile_pool(name="ps_mm", bufs=2, space="PSUM"))
        ps_st = ctx.enter_context(tc.tile_pool(name="ps_st", bufs=2, space="PSUM"))
        ps_y = ctx.enter_context(tc.tile_pool(name="ps_y", bufs=1, space="PSUM"))

        ident = consts.tile([128, 128], BF16, tag="ident")
        nc.sync.dma_start(out=ident[:], in_=Tn["identin"].ap())
        ones1 = consts.tile([1, 128], F32, tag="ones1")
        nc.vector.memset(ones1[:], 1.0)
        onescol = consts.tile([128, 1], F32, tag="onescol")
        nc.vector.memset(onescol[:], 1.0)
        onescol16 = consts.tile([128, 1], BF16, tag="onescol16")
        nc.vector.memset(onescol16[:], 1.0)
        ones_row = consts.tile([1, CH], BF16, tag="ones_row")
        nc.vector.memset(ones_row[:], 1.0)
        nfw_sb = consts.tile([128, NG], F32, tag="nfw")
        nc.sync.dma_start(out=nfw_sb[:], in_=Tn["nfw"].ap())
        nfb_sb = consts.tile([128, NG], F32, tag="nfb")
        nc.sync.dma_start(out=nfb_sb[:], in_=Tn["nfb"].ap())
        eps_sb = consts.tile([128, 1], F32, tag="eps")
        nc.vector.memset(eps_sb[:], EPS)

        def mm(out, lhsT, rhs, **kw):
            nc.tensor.matmul(out, lhsT=lhsT, rhs=rhs, **kw)

        def src_ap(i, b, g, t0, t1):
            """Block-i input (already flipped), batch b, feature group g."""
            if i == 0:
                return Tn["xT"].ap()[128 * g:128 * (g + 1), b * L + t0: b * L + t1]
            return op_out[i - 1][b].ap()[128 * g:128 * (g + 1), t0:t1]

        for i in range(NB):
            # ---------------- per-block weights ----------------
            wi_sb = wpool.tile([128, 1024], F32, tag="wi", bufs=1)
            nc.sync.dma_start(out=wi_sb[:], in_=Tn["wi"].ap()[i])
            negrs_sb = wpool.tile([1, 256], F32, tag="negrs", bufs=1)
            nc.sync.dma_start(out=negrs_sb[:], in_=Tn["negrs"].ap()[i])
            convd_sb = wpool.tile([128, KCONV * 128], BF16, tag="convd", bufs=1)
            nc.sync.dma_start(out=convd_sb[:], in_=Tn["convd"].ap()[i])
            convb_sb = wpool.tile([128, 1], F32, tag="convb")
            nc.sync.dma_start(out=convb_sb[:], in_=Tn["convb"].ap()[i])
            wx_sb = wpool.tile([128, 64], BF16, tag="wx", bufs=1)
            nc.sync.dma_start(out=wx_sb[:], in_=Tn["wxT"].ap()[i])
            wdt_sb = wpool.tile([32, 128], BF16, tag="wdt", bufs=1)
            nc.sync.dma_start(out=wdt_sb[:], in_=Tn["wdtT"].ap()[i])
            bdt_sb = wpool.tile([1, 128], BF16, tag="bdt", bufs=1)
            nc.sync.dma_start(out=bdt_sb[:], in_=Tn["bdt"].ap()[i])
            acols_sb = wpool.tile([128, NST], F32, tag="acols", bufs=1)
            nc.sync.dma_start(out=acols_sb[:], in_=Tn["acols"].ap()[i])
            dpcol_sb = wpool.tile([128, 1], F32, tag="dpcol", bufs=1)
            nc.sync.dma_start(out=dpcol_sb[:], in_=Tn["dpcol"].ap()[i])
            wo_sb = wpool.tile([128, 512], F32, tag="wo", bufs=1)
            nc.sync.dma_start(out=wo_sb[:], in_=Tn["woT"].ap()[i])
            if has_lnb:
                biasin_sb = wpool.tile([128, 2], F32, tag="biasin")
                nc.sync.dma_start(out=biasin_sb[:], in_=Tn["biasin"].ap()[i])

            # ---- fused stats + in-proj + conv, per batch ----
            xipad = [bigs.tile([128, L + 3], BF16, tag=f"xipad{b}",
                               name=f"xipad{b}") for b in range(B)]
            sz = [bigs.tile([128, L], F32, tag=f"sz{b}", name=f"sz{b}")
                  for b in range(B)]
            for b in range(B):
                nc.vector.memset(xipad[b][:, 0:3], 0.0)
            for b in range(B):
                # full-L group tiles, loaded once per (block, batch)
                xg_full = []
                for g in range(NG):
                    xg = xin.tile([128, L], F32, tag=f"xg{g}", bufs=1, name=f"xg{g}")
                    nc.sync.dma_start(out=xg[:], in_=src_ap(i, b, g, 0, L))
                    xg_full.append(xg)
                for c in range(NCH):
                    t0 = c * CH
                    s1_ps = ps_st.tile([1, CH], F32, tag="st")
                    s2_ps = ps_st.tile([1, CH], F32, tag="st")
                    for g in range(NG):
                        xsq = xsqp.tile([128, CH], BF16, tag="xsq",
                                        name="xsq")
                        nc.scalar.square(out=xsq[:],
                                         in_=xg_full[g][:, t0:t0 + CH])
                        mm(s1_ps[:], lhsT=onescol[:],
                           rhs=xg_full[g][:, t0:t0 + CH],
                           start=(g == 0), stop=(g == NG - 1))
                        mm(s2_ps[:], lhsT=onescol16[:],
                           rhs=xsq[:],
                           start=(g == 0), stop=(g == NG - 1))
                    s1_row = stats.tile([1, CH], F32, tag="s1",
                                        name="s1_row")
                    nc.scalar.copy(out=s1_row[:], in_=s1_ps[:])
                    mu2 = small.tile([1, CH], F32, tag="mu2")
                    nc.scalar.activation(out=mu2[:], in_=s1_ps[:],
                                         func=Act.Square, scale=1.0 / D)
                    var_row = stats.tile([1, CH], F32, tag="lnv",
                                         name="lnv_row")
                    nc.vector.scalar_tensor_tensor(
                        out=var_row[:], in0=s2_ps[:],
                        scalar=1.0 / D, in1=mu2[:], op0=Alu.mult,
                        op1=Alu.subtract)
                    nc.scalar.activation(out=var_row[:], in_=var_row[:],
                                         func=Act.Ln, bias=eps_sb[:1, :])
                    rstd_row = small.tile([1, CH], F32, tag="rstds",
                                          name="rstd_row")
                    nc.scalar.activation(out=rstd_row[:], in_=var_row[:],
                                         func=Act.Exp, scale=-0.5)
                    nc.sync.dma_start(
                        out=Tn["rstd_scr"].ap()[b:b + 1, t0:t0 + CH],
                        in_=rstd_row[:])
                    rbc = small.tile([128, CH], F32, tag="rbc")
                    nc.sync.dma_start(
                        out=rbc[:],
                        in_=Tn["rstd_scr"].ap()[b:b + 1, t0:t0 + CH]
                            .partition_broadcast(128))
                    for grp in range(2):  # 0 = xi, 1 = z
                        xz_ps = ps_mm.tile([128, CH], F32, tag="mm")
                        for k in range(4):
                            lh = wi_sb[:, (grp * 4 + k) * 128:
                                       (grp * 4 + k + 1) * 128]
                            mm(xz_ps[:], lhsT=lh,
                               rhs=xg_full[k][:, t0:t0 + CH],
                               start=(k == 0), stop=False)
                        mm(xz_ps[:],
                           lhsT=negrs_sb[:, grp * 128:(grp + 1) * 128],
                           rhs=s1_row[:], start=False, stop=True)
                        if grp == 0:
                            dest = xipad[b][:, 3 + t0: 3 + t0 + CH]
                        else:
                            dest = sz[b][:, t0: t0 + CH]
                        nc.vector.tensor_mul(out=dest, in0=xz_ps[:],
                                             in1=rbc[:])
                        if has_lnb:
                            nc.vector.tensor_scalar_add(
                                out=dest, in0=dest,
                                scalar1=biasin_sb[:, grp:grp + 1])
                    cv_ps = ps_mm.tile([128, CH], F32, tag="mm")
                    for kk in range(KCONV):
                        mm(cv_ps[:],
                           lhsT=convd_sb[:, kk * 128:(kk + 1) * 128],
                           rhs=xipad[b][:, t0 + kk: t0 + kk + CH],
                           start=(kk == 0), stop=(kk == KCONV - 1))
                    nc.scalar.activation(out=xipad[b][:, t0:t0 + CH],
                                         in_=cv_ps[:], func=Act.Identity,
                                         bias=convb_sb[:])
            # ---- silus for both batches (one table switch) ----
            for b in range(B):
                nc.scalar.activation(out=xipad[b][:, 0:L],
                                     in_=xipad[b][:, 0:L],
                                     func=Act.Silu)
                nc.scalar.activation(out=sz[b][:], in_=sz[b][:],
                                     func=Act.Silu)
            # ---- Wx projection + AllReduce per batch ----
            for b in range(B):
                for c in range(NCH):
                    t0 = c * CH
                    wx_ps = ps_mm.tile([64, CH], F32, tag="mm")
                    mm(wx_ps[:], lhsT=wx_sb[:], rhs=xipad[b][:, t0:t0 + CH],
                       start=True, stop=True)
                    wxe = small.tile([64, CH], BF16, tag="mu2", name="wxe")
                    nc.scalar.copy(out=wxe[:], in_=wx_ps[:])
                    nc.sync.dma_start(out=xdbl_in[i][b].ap()[:, t0:t0 + CH],
                                      in_=wxe[:])
                nc.gpsimd.collective_compute(
                    "AllReduce", Alu.add, replica_groups=RG,
                    ins=[xdbl_in[i][b].ap()], outs=[xdbl_out[i][b].ap()])

            # ---- phase D: dt (softplus), full-L scan per state ----
            for b in range(B):
                xdbl_sb = dpool.tile([64, L], BF16, tag="xdbl", bufs=1, name="xdbl")
                nc.sync.dma_start(out=xdbl_sb[:],
                                  in_=xdbl_out[i][b].ap())
                dtraw = dpool.tile([128, L], F32, tag="dtraw", bufs=1, name="dtraw")
                esb = dpool.tile([128, L], F32, tag="esb", bufs=1, name="esb")
                for c in range(NCH):
                    t0 = c * CH
                    dt_ps = ps_mm.tile([128, CH], F32, tag="mm",
                                       name="dt_ps")
                    mm(dt_ps[:], lhsT=wdt_sb[:],
                       rhs=xdbl_sb[0:32, t0:t0 + CH],
                       start=True, stop=False)
                    mm(dt_ps[:], lhsT=bdt_sb[:], rhs=ones_row[:],
                       start=False, stop=True)
                    nc.scalar.activation(out=esb[:, t0:t0 + CH],
                                         in_=dt_ps[:],
                                         func=Act.Exp, scale=-1.0)
                    nc.scalar.copy(out=dtraw[:, t0:t0 + CH], in_=dt_ps[:])
                nc.scalar.activation(out=esb[:], in_=esb[:],
                                     func=Act.Ln, bias=1.0)
                dt16 = dpool.tile([128, L], F32, tag="dt16", bufs=1, name="dt16")
                nc.vector.tensor_add(out=dt16[:], in0=dtraw[:], in1=esb[:])
                dtx = dpool.tile([128, L], BF16, tag="dtx", bufs=1, name="dtx")
                nc.vector.tensor_mul(out=dtx[:], in0=dt16[:],
                                     in1=xipad[b][:, 0:L])

                y_ps = ps_y.tile([128, L], F32, tag="y")
                for n in range(NST):
                    bbc = bcpool.tile([128, L], BF16, tag="bbc", name="bbc")
                    nc.sync.dma_start(
                        out=bbc[:],
                        in_=xdbl_out[i][b].ap()[32 + n:33 + n, :]
                            .partition_broadcast(128))
                    cbc = bcpool.tile([128, L], BF16, tag="cbc", name="cbc")
                    nc.sync.dma_start(
                        out=cbc[:],
                        in_=xdbl_out[i][b].ap()[48 + n:49 + n, :]
                            .partition_broadcast(128))
                    dA = spool.tile([128, L], F32, tag="dA", bufs=2)
                    nc.scalar.activation(out=dA[:], in_=dt16[:],
                                         func=Act.Exp,
                                         scale=acols_sb[:, n:n + 1])
                    dBu = spool.tile([128, L], BF16, tag="dBu", bufs=1)
                    nc.vector.tensor_mul(out=dBu[:], in0=dtx[:], in1=bbc[:])
                    h = spool.tile([128, L], BF16, tag="h", bufs=1)
                    nc.vector.tensor_tensor_scan(h[:], dA[:], dBu[:],
                                                 0.0, op0=Alu.mult,
                                                 op1=Alu.add)
                    yterm = spool.tile([128, L], BF16, tag="yterm", bufs=1)
                    nc.vector.tensor_mul(out=yterm[:], in0=h[:], in1=cbc[:])
                    for c in range(NCH):
                        t0 = c * CH
                        mm(y_ps[:, t0:t0 + CH], lhsT=ident[:],
                           rhs=yterm[:, t0:t0 + CH],
                           start=(n == 0), stop=(n == NST - 1))
                for c in range(NCH):
                    t0 = c * CH
                    yg = evac.tile([128, CH], F32, tag="yg", bufs=1)
                    nc.vector.scalar_tensor_tensor(
                        out=yg[:], in0=xipad[b][:, t0:t0 + CH],
                        scalar=dpcol_sb[:], in1=y_ps[:, t0:t0 + CH],
                        op0=Alu.mult, op1=Alu.add)
                    nc.vector.tensor_mul(out=yg[:], in0=yg[:],
                                         in1=sz[b][:, t0: t0 + CH])
                    ft0 = L - (c + 1) * CH
                    for g in range(NG):
                        op_ps = ps_mm.tile([128, CH], F32, tag="mm")
                        mm(op_ps[:], lhsT=wo_sb[:, g * 128:(g + 1) * 128],
                           rhs=yg[:], start=True, stop=True)
                        og = evac.tile([128, CH], F32, tag="og")
                        nc.scalar.copy(out=og[:, ::-1], in_=op_ps[:])
                        nc.sync.dma_start(
                            out=op_in[i][b].ap()[g * 128:(g + 1) * 128,
                                                 ft0:ft0 + CH],
                            in_=og[:])
                nc.gpsimd.collective_compute(
                    "AllReduce", Alu.add, replica_groups=RG,
                    ins=[op_in[i][b].ap()], outs=[op_out[i][b].ap()])

        # ---------------- final layernorm (token-replicated) ----------------
        for b in range(B):
            for c in range(NCH):
                t0 = c * CH
                xg_tiles = []
                for g in range(NG):
                    xg = xin.tile([128, CH], F32, tag="xgf", bufs=5,
                                  name="xgf")
                    nc.sync.dma_start(
                        out=xg[:],
                        in_=src_ap(NB, b, g, t0, t0 + CH))
                    xg_tiles.append(xg)
                s1_ps = ps_st.tile([1, CH], F32, tag="st")
                s2_ps = ps_st.tile([1, CH], F32, tag="st")
                for g in range(NG):
                    xsq = small.tile([128, CH], BF16, tag="xsqf",
                                     name="xsqf")
                    nc.scalar.square(out=xsq[:], in_=xg_tiles[g][:])
                    mm(s1_ps[:], lhsT=onescol[:], rhs=xg_tiles[g][:],
                       start=(g == 0), stop=(g == NG - 1))
                    mm(s2_ps[:], lhsT=onescol16[:], rhs=xsq[:],
                       start=(g == 0), stop=(g == NG - 1))
                m_row = small.tile([1, CH], F32, tag="m_row")
                nc.vector.tensor_scalar_mul(out=m_row[:], in0=s1_ps[:],
                                            scalar1=1.0 / D)
                mu2 = small.tile([1, CH], F32, tag="mu2f")
                nc.scalar.activation(out=mu2[:], in_=s1_ps[:],
                                     func=Act.Square, scale=1.0 / D)
                var_row = small.tile([1, CH], F32, tag="var")
                nc.vector.scalar_tensor_tensor(
                    out=var_row[:], in0=s2_ps[:], scalar=1.0 / D, in1=mu2[:],
                    op0=Alu.mult, op1=Alu.subtract)
                rstd_row = small.tile([1, CH], F32, tag="rstdf",
                                      name="rstd_row")
                nc.scalar.activation(out=var_row[:], in_=var_row[:],
                                     func=Act.Ln, bias=eps_sb[:1, :])
                nc.scalar.activation(out=rstd_row[:], in_=var_row[:],
                                     func=Act.Exp, scale=-0.5)
                mbc_ps = ps_mm.tile([128, CH], F32, tag="mm")
                mm(mbc_ps[:], lhsT=ones1[:], rhs=m_row[:],
                   start=True, stop=True)
                nc.sync.dma_start(
                    out=Tn["rstd_scr"].ap()[b:b + 1, t0:t0 + CH],
                    in_=rstd_row[:])
                rbc = small.tile([128, CH], F32, tag="rbcf")
                nc.sync.dma_start(
                    out=rbc[:],
                    in_=Tn["rstd_scr"].ap()[b:b + 1, t0:t0 + CH]
                        .partition_broadcast(128))
                for g in range(NG):
                    t1_sb = small.tile([128, CH], F32, tag="t1f",
                                       name="t1_sb")
                    nc.vector.tensor_sub(out=t1_sb[:],
                                         in0=xg_tiles[g][:],
                                         in1=mbc_ps[:])
                    o_sb = evac.tile([128, CH], F32, tag="og", name="o_sb")
                    nc.vector.scalar_tensor_tensor(
                        out=o_sb[:], in0=t1_sb[:], scalar=nfw_sb[:, g:g + 1],
                        in1=rbc[:], op0=Alu.mult, op1=Alu.mult)
                    if has_nfb:
                        nc.vector.tensor_scalar_add(
                            out=o_sb[:], in0=o_sb[:],
                            scalar1=nfb_sb[:, g:g + 1])
                    nc.sync.dma_start(
                        out=Tn["outT"].ap()[g * 128:(g + 1) * 128,
                                            b * L + t0: b * L + t0 + CH],
                        in_=o_sb[:])


def _host_prep(inputs):
    x = np.asarray(inputs["x"], np.float32)
    ln_w = np.asarray(inputs["ln_w"], np.float32)
    ln_b = np.asarray(inputs["ln_b"], np.float32)
    W_in = np.asarray(inputs["W_in"], np.float32)
    conv_w = np.asarray(inputs["conv_w"], np.float32)
    conv_b = np.asarray(inputs["conv_b"], np.float32)
    W_x = np.asarray(inputs["W_x"], np.float32)
    W_dt = np.asarray(inputs["W_dt"], np.float32)
    b_dt = np.asarray(inputs["b_dt"], np.float32)
    A_log = np.asarray(inputs["A_log"], np.float32)
    D_p = np.asarray(inputs["D_p"], np.float32)
    W_out = np.asarray(inputs["W_out"], np.float32)
    normf_w = np.asarray(inputs["normf_w"], np.float32)
    normf_b = np.asarray(inputs["normf_b"], np.float32)

    bf = ml_dtypes.bfloat16
    xT = np.ascontiguousarray(x.transpose(2, 0, 1).reshape(D, T))
    A = -np.exp(A_log)  # (NB, DI, NST)

    in_maps = []
    for k in range(NCORES):
        sl = slice(DS * k, DS * (k + 1))
        wi_arr = np.zeros((NB, 128, 1024), np.float32)
        negrs_arr = np.zeros((NB, 1, 256), np.float32)
        biasin_arr = np.zeros((NB, 128, 2), np.float32)
        convd_arr = np.zeros((NB, 128, KCONV * 128), np.float32)
        convb_arr = np.zeros((NB, 128, 1), np.float32)
        wx_arr = np.zeros((NB, 128, 64), np.float32)
        wdt_arr = np.zeros((NB, 32, 128), np.float32)
        bdt_arr = np.zeros((NB, 1, 128), np.float32)
        acols_arr = np.zeros((NB, 128, NST), np.float32)
        dpcol_arr = np.zeros((NB, 128, 1), np.float32)
        wo_arr = np.zeros((NB, 128, 512), np.float32)
        for i in range(NB):
            Wf = W_in[i] * ln_w[i][None, :]          # (2DI, D)
            rows = [np.arange(DS * k, DS * (k + 1)),
                    np.arange(DI + DS * k, DI + DS * (k + 1))]
            for grp in range(2):
                Wg = Wf[rows[grp], :]                # (128, 512)
                lhsT = Wg.T.reshape(4, 128, 128)     # [kc, p, m]
                for kc in range(4):
                    wi_arr[i, :, (grp * 4 + kc) * 128:(grp * 4 + kc + 1) * 128] = \
                        lhsT[kc]
                negrs_arr[i, 0, grp * 128:(grp + 1) * 128] = -Wg.sum(1) / D
                biasin_arr[i, :, grp] = W_in[i][rows[grp], :] @ ln_b[i]
            for kk in range(KCONV):
                np.fill_diagonal(
                    convd_arr[i, :, kk * 128:(kk + 1) * 128],
                    conv_w[i, sl, kk])
            convb_arr[i, :, 0] = conv_b[i, sl]
            wx_arr[i] = W_x[i][:, sl].T              # (128, 64)
            wdt_arr[i] = W_dt[i][sl, :].T            # (32, 128)
            bdt_arr[i, 0, :] = b_dt[i, sl]
            acols_arr[i] = A[i, sl, :]
            dpcol_arr[i, :, 0] = D_p[i, sl]
            wo_arr[i] = W_out[i][:, sl].T            # (128, 512)
        in_maps.append({
            "xT": xT,
            "wi": wi_arr, "negrs": negrs_arr,
            "biasin": biasin_arr,
            "convd": convd_arr.astype(bf), "convb": convb_arr,
            "wxT": wx_arr.astype(bf), "wdtT": wdt_arr.astype(bf),
            "bdt": bdt_arr.astype(bf),
            "acols": acols_arr, "dpcol": dpcol_arr,
            "woT": wo_arr,
            "nfw": np.ascontiguousarray(normf_w.reshape(NG, 128).T),
            "nfb": np.ascontiguousarray(normf_b.reshape(NG, 128).T),
            "identin": np.eye(128, dtype=np.float32).astype(bf),
        })
    has_lnb = bool(np.any(ln_b != 0.0))
    has_nfb = bool(np.any(normf_b != 0.0))
    return in_maps, has_lnb, has_nfb


def _get_program(has_lnb, has_nfb):
    key = (has_lnb, has_nfb)
    if key not in _PROGRAM_CACHE:
        _PROGRAM_CACHE[key] = _build_program(has_lnb, has_nfb)
    return _PROGRAM_CACHE[key]


def kernel(**inputs) -> np.ndarray:
    in_maps, has_lnb, has_nfb = _host_prep(inputs)
    nc = _get_program(has_lnb, has_nfb)
    res = bass_utils.run_bass_kernel_spmd(nc, in_maps,
                                          core_ids=list(range(NCORES)))
    out_T = res.results[0]["outT"]                   # (512, 4096)
    out = out_T.reshape(D, B, L).transpose(1, 2, 0)  # (B, L, D)
    return np.ascontiguousarray(out.astype(np.float32))
